# revision 40
# baseline (speedup 1.0000x reference)
"""Trainium2 Bass kernel for AttentionalPositionEncoding.

Reference computation (per batch b, with x_tok = x.reshape(C, N).T):
    cnn   = x_tok @ Wc.T
    q     = cnn @ Wq.T + bq           -> heads [h=8, N=1024, dk=32]
    k     = pos @ Wk.T + bk
    v     = pos @ Wv.T + bv
    attn  = softmax(q k^T / sqrt(dk)) @ v
    out   = (cnn @ Wf.T + bf + attn) @ Wo.T + bo + x_tok

Sharding: data-parallel over B=8 across the 8 NeuronCores (1 batch/core).

Host-side weight folding (exact algebra, done in fp32):
    Wqc  = Wq @ Wc          (q     = x_tok @ Wqc.T + bq)
    Wofc = Wo @ Wf @ Wc     (ffn   = x_tok @ Wofc.T)
    bfo  = Wo @ bf + bo

On-chip layout is feature-major ("CN": features on partitions, tokens on
free dim), which matches the HBM layout of x/pos ([C, H*W]) so no input
transposes are needed.  Attention scores are computed *transposed*
(S^T[j, i], keys on psum partitions) so that exp(S^T) feeds the P@V
matmul directly as the moving operand.  Softmax is unnormalized
(scores are O(8), exp is safe in fp32); the row sums Z are produced by a
ones-column appended to V (M=33 stationary), and 1/Z is applied after
P@V via a small select-matrix broadcast matmul.
"""

import math

import numpy as np

import concourse.bacc as bacc
import concourse.mybir as mybir
import concourse.tile as tile
from concourse.bass_utils import run_bass_kernel_spmd

F32 = mybir.dt.float32
F32R = mybir.dt.float32r

D = 256          # d_model
H = 8            # heads
DK = 32          # head dim
N = 1024         # tokens (32*32)
NCORES = 8
SCALE = 1.0 / math.sqrt(DK)


def _r(ap):
    """Bitcast an AP to float32r so the PE runs at 1 cycle/row."""
    return ap.bitcast(F32R)


def build(loop_input=False):
    """Build the per-core Bass program.

    loop_input=True adds a uint32 [1,1] input "niter" and wraps the whole
    body in a dynamic For_i — used by the local timing harness only.
    """
    nc = bacc.Bacc(None, target_bir_lowering=False)

    x_d = nc.dram_tensor("x", [D, N], F32R, kind="ExternalInput")
    pos_d = nc.dram_tensor("pos", [D, N], F32R, kind="ExternalInput")
    wqcT_d = nc.dram_tensor("wqcT", [D, D], F32R, kind="ExternalInput")
    wkT_d = nc.dram_tensor("wkT", [D, D], F32R, kind="ExternalInput")
    # v weights augmented with a zero 33rd column per head; the ones come
    # from the bias row, so P@V also produces the softmax denominators Z.
    wvT_d = nc.dram_tensor("wvT", [D, H * (DK + 1)], F32R, kind="ExternalInput")
    wofcT_d = nc.dram_tensor("wofcT", [D, D], F32R, kind="ExternalInput")
    woT_d = nc.dram_tensor("woT", [D, D], F32R, kind="ExternalInput")
    bpp_d = nc.dram_tensor("b_pp", [128, 6], F32, kind="ExternalInput")
    brow_d = nc.dram_tensor("b_row", [1, H * (DK + 1)], F32R,
                            kind="ExternalInput")
    ones_d = nc.dram_tensor("ones1", [1, 128], F32R, kind="ExternalInput")
    out_d = nc.dram_tensor("out", [D, N], F32, kind="ExternalOutput")
    if loop_input:
        niter_d = nc.dram_tensor("niter", [1, 1], mybir.dt.uint32,
                                 kind="ExternalInput")

    with tile.TileContext(nc) as tc:
        import contextlib
        with contextlib.ExitStack() as stk:
            if loop_input:
                cpool = stk.enter_context(tc.tile_pool(name="cfg", bufs=1))
                nit_sb = cpool.tile([1, 1], mybir.dt.uint32)
                nc.sync.dma_start(nit_sb[:], niter_d[:])
                nit = nc.values_load(nit_sb[0:1, 0:1], min_val=1,
                                     max_val=1 << 20,
                                     skip_runtime_bounds_check=True)
                loop_cm = tc.For_i(0, nit, 1,
                                   hint_engines=tuple(mybir.ALL_ENGINES))
            else:
                loop_cm = contextlib.nullcontext()
            with loop_cm:
                _body(nc, tc, x_d, pos_d, wqcT_d, wkT_d, wvT_d, wofcT_d,
                      woT_d, bpp_d, brow_d, ones_d, out_d)
    nc.compile()
    return nc


def _body(nc, tc, x_d, pos_d, wqcT_d, wkT_d, wvT_d, wofcT_d, woT_d,
          bpp_d, brow_d, ones_d, out_d):
    import contextlib
    with contextlib.ExitStack() as stk:
        ep = stk.enter_context

        persist = ep(tc.tile_pool(name="persist", bufs=1))

        # ---------- load inputs ----------
        def load_cn(dram, name):
            t = persist.tile([128, 2, dram.shape[1]], F32R, tag=name)
            nc.sync.dma_start(t[:], dram[:].rearrange("(k p) n -> p k n", p=128))
            return t

        x_sb = load_cn(x_d, "x_sb")          # [128, kt, 1024]
        pos_sb = load_cn(pos_d, "pos_sb")
        wqc_sb = load_cn(wqcT_d, "wqc_sb")   # [128, kt, 256]
        wk_sb = load_cn(wkT_d, "wk_sb")
        wv_sb = load_cn(wvT_d, "wv_sb")
        wofc_sb = load_cn(wofcT_d, "wofc_sb")
        wo_sb = load_cn(woT_d, "wo_sb")
        bpp = persist.tile([128, 6], F32, tag="bpp")
        nc.sync.dma_start(bpp[:], bpp_d[:])
        brow = persist.tile([1, H * (DK + 1)], F32R, tag="brow")
        nc.sync.dma_start(brow[:], brow_d[:])

        # constants
        ones1 = persist.tile([1, 128], F32R, tag="ones1")
        nc.sync.dma_start(ones1[:], ones_d[:])
        zbias = persist.tile([128, 1], F32, tag="zbias")
        nc.gpsimd.memset(zbias[:], 0.0)

        # persistent activations
        q_sb = persist.tile([128, 2, N], F32R, tag="q_sb")
        k_sb = persist.tile([128, 2, N], F32R, tag="k_sb")
        v_aug = persist.tile([128, 8, H * (DK + 1)], F32R, tag="v_aug")
        oT_sb = persist.tile([128, 2, N], F32R, tag="oT_sb")
        # 1/Z rows, all on partition 0 (engine APs need 32-aligned bases)
        zinv = persist.tile([1, H, N], F32R, tag="zinv")
        out_sb = persist.tile([128, 2, N], F32, tag="out_sb")

        # ---------- q / k projections (CN layout) ----------
        with tc.tile_pool(name="dense_ps", bufs=2, space="PSUM") as dense_ps:
            for (dst, w_sb, rhs_sb, bcol) in ((q_sb, wqc_sb, x_sb, 0),
                                              (k_sb, wk_sb, pos_sb, 2)):
                for mt in range(2):
                    for ch in range(2):
                        ps = dense_ps.tile([128, 512], F32, tag="dense")
                        for kt in range(2):
                            nc.tensor.matmul(
                                ps[:],
                                _r(w_sb[:, kt, 128 * mt:128 * mt + 128]),
                                _r(rhs_sb[:, kt, 512 * ch:512 * ch + 512]),
                                start=(kt == 0), stop=(kt == 1))
                        nc.vector.tensor_scalar_add(
                            dst[:, mt, 512 * ch:512 * ch + 512], ps[:],
                            bpp[:, bcol + mt:bcol + mt + 1])

            # ------- v projection (token-major, ones-augmented) -------
            for jt in range(8):
                ps = dense_ps.tile([128, H * (DK + 1)], F32, tag="dense")
                for kt in range(2):
                    nc.tensor.matmul(
                        ps[:],
                        _r(pos_sb[:, kt, 128 * jt:128 * jt + 128]),
                        _r(wv_sb[:, kt, :]),
                        start=(kt == 0), stop=False)
                nc.tensor.matmul(ps[:], _r(ones1[:]), _r(brow[:]),
                                 start=False, stop=True)
                nc.vector.tensor_copy(v_aug[:, jt, :], ps[:])

        # ---------- attention: head pairs ----------
        attn_stk = stk.enter_context(contextlib.ExitStack())
        sc_ps = attn_stk.enter_context(
            tc.tile_pool(name="sc_ps", bufs=2, space="PSUM"))
        pv_ps = attn_stk.enter_context(
            tc.tile_pool(name="pv_ps", bufs=1, space="PSUM"))
        e_pool = attn_stk.enter_context(tc.tile_pool(name="e_pool", bufs=3))

        for hp in range(4):
            hA, hB = 2 * hp, 2 * hp + 1
            dt = hA // 4
            pA, pB = 32 * (hA % 4), 32 * (hB % 4)
            # fp32r matmul dst base partition must be 0 (or 64 with M<=32):
            # give each head its own psum tile, both written at (0, 0), M=33.
            pvA = pv_ps.tile([128, N], F32, tag="pvA")
            pvB = pv_ps.tile([128, N], F32, tag="pvB")
            for jt in range(8):
                for ch in range(2):
                    sc = sc_ps.tile([128, 1024], F32, tag="sc")
                    # S^T tiles for heads A (cols 0:512) and B (cols 512:1024)
                    for (h0, p0, lo) in ((hA, pA, 0), (hB, pB, 512)):
                        nc.tensor.matmul(
                            sc[:, lo:lo + 512],
                            _r(k_sb[p0:p0 + 32, dt, 128 * jt:128 * jt + 128]),
                            _r(q_sb[p0:p0 + 32, dt, 512 * ch:512 * ch + 512]),
                            start=True, stop=True, tile_position=(p0, 0))
                    et = e_pool.tile([128, 1024], F32R, tag="et")
                    nc.scalar.activation(et[:], sc[:],
                                         mybir.ActivationFunctionType.Exp,
                                         bias=zbias[:, 0:1], scale=SCALE)
                    # P@V (+Z row at 32): accumulate over jt
                    for (h0, elo, pvt) in ((hA, 0, pvA), (hB, 512, pvB)):
                        nc.tensor.matmul(
                            pvt[0:DK + 1, 512 * ch:512 * ch + 512],
                            _r(v_aug[:, jt,
                                     (DK + 1) * h0:(DK + 1) * h0 + DK + 1]),
                            _r(et[:, elo:elo + 512]),
                            start=(jt == 0), stop=(jt == 7),
                            tile_position=(0, 0))
            # evacuate: unnormalized attn^T (CN) + 1/Z rows
            nc.vector.tensor_copy(oT_sb[pA:pA + 32, dt, :], pvA[0:32, :])
            nc.vector.tensor_copy(oT_sb[pB:pB + 32, dt, :], pvB[0:32, :])
            with nc.allow_low_precision(reason="f32r is full 32-bit width"):
                nc.vector.reciprocal(zinv[0:1, hA, :], pvA[32:33, :])
                nc.vector.reciprocal(zinv[0:1, hB, :], pvB[32:33, :])

        attn_stk.close()

        # ---------- normalize attn^T by 1/Z (K=1 broadcast matmuls) ----------
        # dst base partition must be 0, so broadcast each head-group row into
        # [32, g, 512] psum and multiply with a partition-shifted DVE op.
        z_ps = ep(tc.tile_pool(name="z_ps", bufs=1, space="PSUM"))
        for dt in range(2):
            for ch in range(2):
                zx = z_ps.tile([32, 4, 512], F32, tag="zx")
                for g in range(4):
                    nc.tensor.matmul(
                        zx[0:32, g, :],
                        _r(ones1[0:1, 0:32]),
                        _r(zinv[0:1, 4 * dt + g, 512 * ch:512 * ch + 512]),
                        start=True, stop=True, tile_position=(0, 0))
                for g in range(4):
                    sl = (slice(32 * g, 32 * g + 32), dt,
                          slice(512 * ch, 512 * ch + 512))
                    nc.vector.tensor_mul(oT_sb[sl], oT_sb[sl], zx[0:32, g, :])

        # ---------- output: Wo @ attn^T + Wofc @ x^T + bfo + x ----------
        fin_ps = ep(tc.tile_pool(name="fin_ps", bufs=2, space="PSUM"))
        for ct in range(2):
            for ch in range(2):
                ps = fin_ps.tile([128, 512], F32, tag="fin")
                first = True
                for (w_sb, rhs_sb) in ((wo_sb, oT_sb), (wofc_sb, x_sb)):
                    for kt in range(2):
                        nc.tensor.matmul(
                            ps[:],
                            _r(w_sb[:, kt, 128 * ct:128 * ct + 128]),
                            _r(rhs_sb[:, kt, 512 * ch:512 * ch + 512]),
                            start=first, stop=(w_sb is wofc_sb and kt == 1))
                        first = False
                sl = (slice(None), ct, slice(512 * ch, 512 * ch + 512))
                nc.vector.tensor_add(out_sb[sl], ps[:],
                                     x_sb[:, ct, 512 * ch:512 * ch + 512])
                nc.vector.tensor_scalar_add(out_sb[sl], out_sb[sl],
                                            bpp[:, 4 + ct:4 + ct + 1])
        nc.sync.dma_start(out_d[:].rearrange("(k p) n -> p k n", p=128),
                          out_sb[:])


_CACHE = {}


def _get_nc(loop_input=False):
    if loop_input not in _CACHE:
        _CACHE[loop_input] = build(loop_input)
    return _CACHE[loop_input]


def make_in_maps(x, pos_code, Wq, bq, Wk, bk, Wv, bv, Wo, bo, Wc, Wf, bf,
                 extra=None):
    x = np.asarray(x, np.float32)
    pos_code = np.asarray(pos_code, np.float32)
    wqcT = np.ascontiguousarray((np.asarray(Wq) @ np.asarray(Wc)).T, np.float32)
    wkT = np.ascontiguousarray(np.asarray(Wk).T, np.float32)
    # augmented V: per head 32 value cols + a zero col (ones come from bias)
    wvT = np.zeros((D, H * (DK + 1)), np.float32)
    brow = np.zeros((1, H * (DK + 1)), np.float32)
    vT = np.asarray(Wv).T
    bv_np = np.asarray(bv, np.float32)
    for h in range(H):
        wvT[:, (DK + 1) * h:(DK + 1) * h + DK] = vT[:, DK * h:DK * h + DK]
        brow[0, (DK + 1) * h:(DK + 1) * h + DK] = bv_np[DK * h:DK * h + DK]
        brow[0, (DK + 1) * h + DK] = 1.0
    wofcT = np.ascontiguousarray(
        (np.asarray(Wo) @ np.asarray(Wf) @ np.asarray(Wc)).T, np.float32)
    woT = np.ascontiguousarray(np.asarray(Wo).T, np.float32)
    bfo = (np.asarray(Wo) @ np.asarray(bf) + np.asarray(bo)).astype(np.float32)
    b_pp = np.stack([np.asarray(bq, np.float32).reshape(2, 128)[0],
                     np.asarray(bq, np.float32).reshape(2, 128)[1],
                     np.asarray(bk, np.float32).reshape(2, 128)[0],
                     np.asarray(bk, np.float32).reshape(2, 128)[1],
                     bfo.reshape(2, 128)[0],
                     bfo.reshape(2, 128)[1]], axis=1)
    b_pp = np.ascontiguousarray(b_pp, np.float32)          # [128, 6]

    B = x.shape[0]
    in_maps = []
    for b in range(B):
        m = {
            "x": np.ascontiguousarray(x[b].reshape(D, N)),
            "pos": np.ascontiguousarray(pos_code[b].reshape(D, N)),
            "wqcT": wqcT, "wkT": wkT, "wvT": wvT, "wofcT": wofcT,
            "woT": woT, "b_pp": b_pp, "b_row": brow,
            "ones1": np.ones((1, 128), np.float32),
        }
        if extra:
            m.update(extra)
        in_maps.append(m)
    return in_maps


def kernel(**inputs):
    nc = _get_nc(False)
    in_maps = make_in_maps(**inputs)
    res = run_bass_kernel_spmd(nc, in_maps, core_ids=list(range(NCORES)),
                               trace=False)
    out = np.stack([r["out"].reshape(D, N).T for r in res.results], axis=0)
    return np.ascontiguousarray(out, np.float32)


# revision 52
# speedup vs baseline: 1.1219x; 1.1219x over previous
"""Trainium2 Bass kernel for AttentionalPositionEncoding.

Reference computation (per batch b, with x_tok = x.reshape(C, N).T):
    cnn   = x_tok @ Wc.T
    q     = cnn @ Wq.T + bq           -> heads [h=8, N=1024, dk=32]
    k     = pos @ Wk.T + bk
    v     = pos @ Wv.T + bv
    attn  = softmax(q k^T / sqrt(dk)) @ v
    out   = (cnn @ Wf.T + bf + attn) @ Wo.T + bo + x_tok

Sharding: data-parallel over B=8 across the 8 NeuronCores (1 batch/core).

Host-side weight folding (exact algebra, done in fp32):
    Wqc  = Wq @ Wc          (q     = x_tok @ Wqc.T + bq)
    Wofc = Wo @ Wf @ Wc     (ffn   = x_tok @ Wofc.T)
    bfo  = Wo @ bf + bo

On-chip layout is feature-major ("CN": features on partitions, tokens on
free dim), which matches the HBM layout of x/pos ([C, H*W]) so no input
transposes are needed.  Attention scores are computed *transposed*
(S^T[j, i], keys on psum partitions) so that exp(S^T) feeds the P@V
matmul directly as the moving operand.  Softmax is unnormalized
(scores are O(8), exp is safe in fp32); the row sums Z are produced by a
ones-column appended to V (M=33 stationary), and 1/Z is applied after
P@V via a small select-matrix broadcast matmul.
"""

import math

import numpy as np

import concourse.bacc as bacc
import concourse.mybir as mybir
import concourse.tile as tile
from concourse.bass_utils import run_bass_kernel_spmd

F32 = mybir.dt.float32
F32R = mybir.dt.float32r
BF16 = mybir.dt.bfloat16

D = 256          # d_model
H = 8            # heads
DK = 32          # head dim
N = 1024         # tokens (32*32)
NCORES = 8
SCALE = 1.0 / math.sqrt(DK)


def _r(ap):
    """Bitcast an AP to float32r so the PE runs at 1 cycle/row."""
    return ap.bitcast(F32R)


def build(loop_input=False, variant="full"):
    """Build the per-core Bass program.

    loop_input=True adds a uint32 [1,1] input "niter" and wraps the whole
    body in a dynamic For_i — used by the local timing harness only.
    variant: "full" | ablations used for local perf attribution.
    """
    nc = bacc.Bacc(None, target_bir_lowering=False)

    x_d = nc.dram_tensor("x", [D, N], F32R, kind="ExternalInput")
    pos_d = nc.dram_tensor("pos", [D, N], F32R, kind="ExternalInput")
    wqcT_d = nc.dram_tensor("wqcT", [D, D], F32R, kind="ExternalInput")
    wkT_d = nc.dram_tensor("wkT", [D, D], F32R, kind="ExternalInput")
    # v weights augmented with a zero 33rd column per head; the ones come
    # from the bias row, so P@V also produces the softmax denominators Z.
    wvT_d = nc.dram_tensor("wvT", [D, H * (DK + 1)], F32R, kind="ExternalInput")
    wofcT_d = nc.dram_tensor("wofcT", [D, D], F32R, kind="ExternalInput")
    woT_d = nc.dram_tensor("woT", [D, D], F32R, kind="ExternalInput")
    bpp_d = nc.dram_tensor("b_pp", [128, 6], F32, kind="ExternalInput")
    brow_d = nc.dram_tensor("b_row", [1, H * (DK + 1)], F32R,
                            kind="ExternalInput")
    ones_d = nc.dram_tensor("ones1", [1, 128], F32R, kind="ExternalInput")
    out_d = nc.dram_tensor("out", [D, N], F32, kind="ExternalOutput")
    if loop_input:
        niter_d = nc.dram_tensor("niter", [1, 1], mybir.dt.uint32,
                                 kind="ExternalInput")

    with tile.TileContext(nc) as tc:
        import contextlib
        with contextlib.ExitStack() as stk:
            if loop_input:
                cpool = stk.enter_context(tc.tile_pool(name="cfg", bufs=1))
                nit_sb = cpool.tile([1, 1], mybir.dt.uint32)
                nc.sync.dma_start(nit_sb[:], niter_d[:])
                nit = nc.values_load(nit_sb[0:1, 0:1], min_val=1,
                                     max_val=1 << 20,
                                     skip_runtime_bounds_check=True)
                loop_cm = tc.For_i(0, nit, 1,
                                   hint_engines=tuple(mybir.ALL_ENGINES))
            else:
                loop_cm = contextlib.nullcontext()
            with loop_cm:
                _body(nc, tc, x_d, pos_d, wqcT_d, wkT_d, wvT_d, wofcT_d,
                      woT_d, bpp_d, brow_d, ones_d, out_d, variant)
    nc.compile()
    return nc


def _body(nc, tc, x_d, pos_d, wqcT_d, wkT_d, wvT_d, wofcT_d, woT_d,
          bpp_d, brow_d, ones_d, out_d, variant="full"):
    import contextlib
    with contextlib.ExitStack() as stk:
        ep = stk.enter_context

        persist = ep(tc.tile_pool(name="persist", bufs=1))

        # ---------- load inputs ----------
        def load_cn(dram, name):
            t = persist.tile([128, 2, dram.shape[1]], F32R, tag=name)
            nc.sync.dma_start(t[:], dram[:].rearrange("(k p) n -> p k n", p=128))
            return t

        x_sb = load_cn(x_d, "x_sb")          # [128, kt, 1024]
        pos_sb = load_cn(pos_d, "pos_sb")
        wqc_sb = load_cn(wqcT_d, "wqc_sb")   # [128, kt, 256]
        wk_sb = load_cn(wkT_d, "wk_sb")
        wv_sb = load_cn(wvT_d, "wv_sb")
        wofc_sb = load_cn(wofcT_d, "wofc_sb")
        wo_sb = load_cn(woT_d, "wo_sb")
        bpp = persist.tile([128, 6], F32, tag="bpp")
        nc.sync.dma_start(bpp[:], bpp_d[:])
        brow = persist.tile([1, H * (DK + 1)], F32R, tag="brow")
        nc.sync.dma_start(brow[:], brow_d[:])

        # constants
        ones1 = persist.tile([1, 128], F32R, tag="ones1")
        nc.sync.dma_start(ones1[:], ones_d[:])
        zbias = persist.tile([128, 1], F32, tag="zbias")
        nc.gpsimd.memset(zbias[:], 0.0)

        # persistent activations
        v2 = variant.startswith("v2")
        pv_dt = BF16 if ("bf16pv" in variant or v2) else F32R
        qk_dt = BF16 if v2 else F32R
        q_sb = persist.tile([128, 2, N], qk_dt, tag="q_sb")
        k_sb = persist.tile([128, 2, N], qk_dt, tag="k_sb")
        v_aug = persist.tile([128, 8, H * (DK + 1)], pv_dt, tag="v_aug")
        oT_sb = persist.tile([128, 2, N], F32R, tag="oT_sb")
        # 1/Z rows, all on partition 0 (engine APs need 32-aligned bases)
        zinv = persist.tile([1, H, N], F32R, tag="zinv")
        out_sb = persist.tile([128, 2, N], F32, tag="out_sb")

        # ---------- q / k projections (CN layout) ----------
        with tc.tile_pool(name="dense_ps", bufs=2, space="PSUM") as dense_ps:
            for (dst, w_sb, rhs_sb, bcol) in ((q_sb, wqc_sb, x_sb, 0),
                                              (k_sb, wk_sb, pos_sb, 2)):
                for mt in range(2):
                    for ch in range(2):
                        ps = dense_ps.tile([128, 512], F32, tag="dense")
                        for kt in range(2):
                            nc.tensor.matmul(
                                ps[:],
                                _r(w_sb[:, kt, 128 * mt:128 * mt + 128]),
                                _r(rhs_sb[:, kt, 512 * ch:512 * ch + 512]),
                                start=(kt == 0), stop=(kt == 1))
                        with nc.allow_low_precision(reason="qk dtype knob"):
                            nc.vector.tensor_scalar_add(
                                dst[:, mt, 512 * ch:512 * ch + 512], ps[:],
                                bpp[:, bcol + mt:bcol + mt + 1])

            # ------- v projection (token-major, ones-augmented) -------
            for jt in range(8):
                ps = dense_ps.tile([128, H * (DK + 1)], F32, tag="dense")
                for kt in range(2):
                    nc.tensor.matmul(
                        ps[:],
                        _r(pos_sb[:, kt, 128 * jt:128 * jt + 128]),
                        _r(wv_sb[:, kt, :]),
                        start=(kt == 0), stop=False)
                nc.tensor.matmul(ps[:], _r(ones1[:]), _r(brow[:]),
                                 start=False, stop=True)
                with nc.allow_low_precision(reason="pv dtype knob"):
                    nc.vector.tensor_copy(v_aug[:, jt, :], ps[:])

        # ---------- attention: head pairs ----------
        attn_stk = stk.enter_context(contextlib.ExitStack())
        sc_ps = attn_stk.enter_context(
            tc.tile_pool(name="sc_ps", bufs=2, space="PSUM"))
        pv_ps = attn_stk.enter_context(
            tc.tile_pool(name="pv_ps", bufs=1, space="PSUM"))
        n_et = 18 if (variant.startswith("batched")
                      or variant.startswith("v2")) else 3
        e_pool = attn_stk.enter_context(tc.tile_pool(name="e_pool", bufs=n_et))

        if variant.startswith("v2"):
            # bf16 attention: per-head score tiles with N=1024 streams.
            for hp in range(4):
                hA, hB = 2 * hp, 2 * hp + 1
                dt = hA // 4
                pA, pB = 32 * (hA % 4), 32 * (hB % 4)
                pvA = pv_ps.tile([128, N], F32, tag="pvA")
                pvB = pv_ps.tile([128, N], F32, tag="pvB")
                ets = {}
                # phase 1: scores + exp, one PE mode (32-row tiles)
                for jt in range(8):
                    for (h0, p0) in ((hA, pA), (hB, pB)):
                        sc = sc_ps.tile([128, 1024], F32, tag="sc")
                        nc.tensor.matmul(
                            sc[:],
                            k_sb[p0:p0 + 32, dt, 128 * jt:128 * jt + 128],
                            q_sb[p0:p0 + 32, dt, :],
                            start=True, stop=True, tile_position=(p0, 0))
                        et = e_pool.tile([128, 1024], BF16, tag="et")
                        with nc.allow_low_precision(reason="bf16 attention"):
                            nc.scalar.activation(
                                et[:], sc[:],
                                mybir.ActivationFunctionType.Exp,
                                bias=zbias[:, 0:1], scale=SCALE)
                        ets[(h0, jt)] = et
                # phase 2: P@V, one PE mode, contiguous accum chains
                for (h0, pvt) in ((hA, pvA), (hB, pvB)):
                    for jt in range(8):
                        nc.tensor.matmul(
                            pvt[0:DK + 1, :],
                            v_aug[:, jt,
                                  (DK + 1) * h0:(DK + 1) * h0 + DK + 1],
                            ets[(h0, jt)][:],
                            start=(jt == 0), stop=(jt == 7),
                            tile_position=(0, 0))
                nc.vector.tensor_copy(oT_sb[pA:pA + 32, dt, :], pvA[0:32, :])
                nc.vector.tensor_copy(oT_sb[pB:pB + 32, dt, :], pvB[0:32, :])
                with nc.allow_low_precision(reason="f32r full 32-bit width"):
                    nc.vector.reciprocal(zinv[0:1, hA, :], pvA[32:33, :])
                    nc.vector.reciprocal(zinv[0:1, hB, :], pvB[32:33, :])

        if variant.startswith("batched"):
            for hp in range(4):
                hA, hB = 2 * hp, 2 * hp + 1
                dt = hA // 4
                pA, pB = 32 * (hA % 4), 32 * (hB % 4)
                pvA = pv_ps.tile([128, N], F32, tag="pvA")
                pvB = pv_ps.tile([128, N], F32, tag="pvB")
                ets = []
                # phase 1: all scores + exp for the pair (one PE mode)
                for jt in range(8):
                    for ch in range(2):
                        sc = sc_ps.tile([128, 1024], F32, tag="sc")
                        for (h0, p0, lo) in ((hA, pA, 0), (hB, pB, 512)):
                            nc.tensor.matmul(
                                sc[:, lo:lo + 512],
                                _r(k_sb[p0:p0 + 32, dt,
                                        128 * jt:128 * jt + 128]),
                                _r(q_sb[p0:p0 + 32, dt,
                                        512 * ch:512 * ch + 512]),
                                start=True, stop=True, tile_position=(p0, 0))
                        et = e_pool.tile([128, 1024], pv_dt, tag="et")
                        with nc.allow_low_precision(reason="pv dtype knob"):
                            nc.scalar.activation(
                                et[:], sc[:],
                                mybir.ActivationFunctionType.Exp,
                                bias=zbias[:, 0:1], scale=SCALE)
                        ets.append(et)
                # phase 2: all P@V for the pair (one PE mode, each psum
                # region's accumulation chain contiguous: BEGIN..MID..END)
                for (h0, elo, pvt) in ((hA, 0, pvA), (hB, 512, pvB)):
                    for ch in range(2):
                        for jt in range(8):
                            et = ets[jt * 2 + ch]
                            grp = ((jt == 0), (jt == 7))
                            if variant == "batched_nogroup":
                                grp = (True, True)
                            nc.tensor.matmul(
                                pvt[0:DK + 1, 512 * ch:512 * ch + 512],
                                v_aug[:, jt,
                                      (DK + 1) * h0:(DK + 1) * h0 + DK + 1],
                                et[:, elo:elo + 512],
                                start=grp[0], stop=grp[1],
                                tile_position=(0, 0))
                nc.vector.tensor_copy(oT_sb[pA:pA + 32, dt, :], pvA[0:32, :])
                nc.vector.tensor_copy(oT_sb[pB:pB + 32, dt, :], pvB[0:32, :])
                with nc.allow_low_precision(reason="f32r full 32-bit width"):
                    nc.vector.reciprocal(zinv[0:1, hA, :], pvA[32:33, :])
                    nc.vector.reciprocal(zinv[0:1, hB, :], pvB[32:33, :])

        for hp in (() if (variant.startswith("batched") or v2) else range(4)):
            hA, hB = 2 * hp, 2 * hp + 1
            dt = hA // 4
            pA, pB = 32 * (hA % 4), 32 * (hB % 4)
            # fp32r matmul dst base partition must be 0 (or 64 with M<=32):
            # give each head its own psum tile, both written at (0, 0), M=33.
            pvA = pv_ps.tile([128, N], F32, tag="pvA")
            pvB = pv_ps.tile([128, N], F32, tag="pvB")
            for jt in range(8):
                for ch in range(2):
                    sc = sc_ps.tile([128, 1024], F32, tag="sc")
                    # S^T tiles for heads A (cols 0:512) and B (cols 512:1024)
                    for (h0, p0, lo) in ((hA, pA, 0), (hB, pB, 512)):
                        nc.tensor.matmul(
                            sc[:, lo:lo + 512],
                            _r(k_sb[p0:p0 + 32, dt, 128 * jt:128 * jt + 128]),
                            _r(q_sb[p0:p0 + 32, dt, 512 * ch:512 * ch + 512]),
                            start=True, stop=True, tile_position=(p0, 0))
                    et = e_pool.tile([128, 1024], F32R, tag="et")
                    if variant == "expdve":
                        with nc.allow_low_precision(reason="perf ablation"):
                            nc.vector.tensor_copy(et[:], sc[:])
                    elif variant == "exp_sbuf":
                        st = e_pool.tile([128, 1024], F32, tag="st")
                        with nc.allow_low_precision(reason="perf ablation"):
                            nc.vector.tensor_copy(st[:], sc[:])
                        nc.scalar.activation(et[:], st[:],
                                             mybir.ActivationFunctionType.Exp,
                                             bias=zbias[:, 0:1], scale=SCALE)
                    else:
                        nc.scalar.activation(et[:], sc[:],
                                             mybir.ActivationFunctionType.Exp,
                                             bias=zbias[:, 0:1], scale=SCALE)
                    # P@V (+Z row at 32): accumulate over jt
                    if variant == "sconly":
                        if jt == 0:
                            for pvt in (pvA, pvB):
                                nc.tensor.matmul(
                                    pvt[0:DK + 1, 512 * ch:512 * ch + 512],
                                    _r(v_aug[:, jt, 0:DK + 1]),
                                    _r(et[:, 0:512]),
                                    start=True, stop=True,
                                    tile_position=(0, 0))
                    else:
                        for (h0, elo, pvt) in ((hA, 0, pvA), (hB, 512, pvB)):
                            grp = ((jt == 0), (jt == 7))
                            if variant == "batched_nogroup":
                                grp = (True, True)
                            nc.tensor.matmul(
                                pvt[0:DK + 1, 512 * ch:512 * ch + 512],
                                v_aug[:, jt,
                                      (DK + 1) * h0:(DK + 1) * h0 + DK + 1],
                                et[:, elo:elo + 512],
                                start=grp[0], stop=grp[1],
                                tile_position=(0, 0))
            # evacuate: unnormalized attn^T (CN) + 1/Z rows
            nc.vector.tensor_copy(oT_sb[pA:pA + 32, dt, :], pvA[0:32, :])
            nc.vector.tensor_copy(oT_sb[pB:pB + 32, dt, :], pvB[0:32, :])
            with nc.allow_low_precision(reason="f32r is full 32-bit width"):
                nc.vector.reciprocal(zinv[0:1, hA, :], pvA[32:33, :])
                nc.vector.reciprocal(zinv[0:1, hB, :], pvB[32:33, :])

        attn_stk.close()

        # ---------- normalize attn^T by 1/Z (K=1 broadcast matmuls) ----------
        # dst base partition must be 0, so broadcast each head-group row into
        # [32, g, 512] psum and multiply with a partition-shifted DVE op.
        z_ps = ep(tc.tile_pool(name="z_ps", bufs=1, space="PSUM"))
        for dt in range(2):
            for ch in range(2):
                zx = z_ps.tile([32, 4, 512], F32, tag="zx")
                for g in range(4):
                    nc.tensor.matmul(
                        zx[0:32, g, :],
                        _r(ones1[0:1, 0:32]),
                        _r(zinv[0:1, 4 * dt + g, 512 * ch:512 * ch + 512]),
                        start=True, stop=True, tile_position=(0, 0))
                for g in range(4):
                    sl = (slice(32 * g, 32 * g + 32), dt,
                          slice(512 * ch, 512 * ch + 512))
                    nc.vector.tensor_mul(oT_sb[sl], oT_sb[sl], zx[0:32, g, :])

        # ---------- output: Wo @ attn^T + Wofc @ x^T + bfo + x ----------
        fin_ps = ep(tc.tile_pool(name="fin_ps", bufs=2, space="PSUM"))
        for ct in range(2):
            for ch in range(2):
                ps = fin_ps.tile([128, 512], F32, tag="fin")
                first = True
                for (w_sb, rhs_sb) in ((wo_sb, oT_sb), (wofc_sb, x_sb)):
                    for kt in range(2):
                        nc.tensor.matmul(
                            ps[:],
                            _r(w_sb[:, kt, 128 * ct:128 * ct + 128]),
                            _r(rhs_sb[:, kt, 512 * ch:512 * ch + 512]),
                            start=first, stop=(w_sb is wofc_sb and kt == 1))
                        first = False
                sl = (slice(None), ct, slice(512 * ch, 512 * ch + 512))
                nc.vector.tensor_add(out_sb[sl], ps[:],
                                     x_sb[:, ct, 512 * ch:512 * ch + 512])
                nc.vector.tensor_scalar_add(out_sb[sl], out_sb[sl],
                                            bpp[:, 4 + ct:4 + ct + 1])
        nc.sync.dma_start(out_d[:].rearrange("(k p) n -> p k n", p=128),
                          out_sb[:])


_CACHE = {}


def _get_nc(loop_input=False, variant="full"):
    key = (loop_input, variant)
    if key not in _CACHE:
        _CACHE[key] = build(loop_input, variant)
    return _CACHE[key]


def make_in_maps(x, pos_code, Wq, bq, Wk, bk, Wv, bv, Wo, bo, Wc, Wf, bf,
                 extra=None):
    x = np.asarray(x, np.float32)
    pos_code = np.asarray(pos_code, np.float32)
    wqcT = np.ascontiguousarray((np.asarray(Wq) @ np.asarray(Wc)).T, np.float32)
    wkT = np.ascontiguousarray(np.asarray(Wk).T, np.float32)
    # augmented V: per head 32 value cols + a zero col (ones come from bias)
    wvT = np.zeros((D, H * (DK + 1)), np.float32)
    brow = np.zeros((1, H * (DK + 1)), np.float32)
    vT = np.asarray(Wv).T
    bv_np = np.asarray(bv, np.float32)
    for h in range(H):
        wvT[:, (DK + 1) * h:(DK + 1) * h + DK] = vT[:, DK * h:DK * h + DK]
        brow[0, (DK + 1) * h:(DK + 1) * h + DK] = bv_np[DK * h:DK * h + DK]
        brow[0, (DK + 1) * h + DK] = 1.0
    wofcT = np.ascontiguousarray(
        (np.asarray(Wo) @ np.asarray(Wf) @ np.asarray(Wc)).T, np.float32)
    woT = np.ascontiguousarray(np.asarray(Wo).T, np.float32)
    bfo = (np.asarray(Wo) @ np.asarray(bf) + np.asarray(bo)).astype(np.float32)
    b_pp = np.stack([np.asarray(bq, np.float32).reshape(2, 128)[0],
                     np.asarray(bq, np.float32).reshape(2, 128)[1],
                     np.asarray(bk, np.float32).reshape(2, 128)[0],
                     np.asarray(bk, np.float32).reshape(2, 128)[1],
                     bfo.reshape(2, 128)[0],
                     bfo.reshape(2, 128)[1]], axis=1)
    b_pp = np.ascontiguousarray(b_pp, np.float32)          # [128, 6]

    B = x.shape[0]
    in_maps = []
    for b in range(B):
        m = {
            "x": np.ascontiguousarray(x[b].reshape(D, N)),
            "pos": np.ascontiguousarray(pos_code[b].reshape(D, N)),
            "wqcT": wqcT, "wkT": wkT, "wvT": wvT, "wofcT": wofcT,
            "woT": woT, "b_pp": b_pp, "b_row": brow,
            "ones1": np.ones((1, 128), np.float32),
        }
        if extra:
            m.update(extra)
        in_maps.append(m)
    return in_maps


def kernel(**inputs):
    nc = _get_nc(False, "batched")
    in_maps = make_in_maps(**inputs)
    res = run_bass_kernel_spmd(nc, in_maps, core_ids=list(range(NCORES)),
                               trace=False)
    out = np.stack([r["out"].reshape(D, N).T for r in res.results], axis=0)
    return np.ascontiguousarray(out, np.float32)
